# revision 1
# baseline (speedup 1.0000x reference)
"""MHSA (dense transformer, ALiBi + causal) TRN2 Bass kernel, 8-core SPMD.

Sharding: batch (2) x head-group (4 groups of 4 heads) -> 8 cores.
Each core computes, for its batch b and heads [4g, 4g+4):
  Q^T = (Wq_slice/8) @ X_q^T      [64+aug, 2048] per head   (f32r matmuls)
  K^T = Wk_slice @ X_k^T          [64+aug, 2048] per head
  V   = X_v @ Wv_slice^T          [2048, 64(+ones)] per head
  S^T = K~ @ Q~^T  with ALiBi folded in via 2 augmented contraction rows:
        Q~ = [Q, slope_h, -slope_h*i],  K~ = [K, j, 1]
  P~^T = exp(S^T - 12) (softmax shift is row-invariant; -12 guards overflow)
  causal: upper-triangle blocks skipped structurally; diagonal 128-strips
        masked by multiplying with a lower-triangular 0/1 tile.
  O^T  = V~^T @ P~^T ; V~ has a ones column so row 64 of O^T is the softmax
        denominator. PE-transpose O^T chunks, divide via scalar-engine copy
        with per-partition reciprocal scale, DMA out.

All matmul operands are float32r (full f32 bytes, 1 cyc/row on PE for N>=256).
"""

import numpy as np

import concourse.bass as bass
import concourse.mybir as mybir
import concourse.tile as tile
from concourse import bacc
from concourse.bass_utils import run_bass_kernel_spmd
from concourse.masks import make_identity

P = 128
S = 2048
D = 1024
H = 16
HWID = 64
HPC = 4           # heads per core
CW = HPC * HWID   # 256 output cols per core
NTSB = 8          # t super-blocks for projection streaming
TSB = S // NTSB   # 256
NKC = D // P      # 8 contraction chunks
NI = S // 512     # 4 i-chunks of 512
NJ = S // P       # 16 j-chunks of 128

F32 = mybir.dt.float32
F32R = mybir.dt.float32r

EXP_BIAS = -12.0


def build_kernel():
    nc = bacc.Bacc("TRN2")

    qT = nc.dram_tensor("qT", [D, S], F32R, kind="ExternalInput")
    kT = nc.dram_tensor("kT", [D, S], F32R, kind="ExternalInput")
    vT = nc.dram_tensor("vT", [D, S], F32R, kind="ExternalInput")
    wqT = nc.dram_tensor("wqT", [D, CW], F32R, kind="ExternalInput")
    wkT = nc.dram_tensor("wkT", [D, CW], F32R, kind="ExternalInput")
    wvT = nc.dram_tensor("wvT", [D, CW], F32R, kind="ExternalInput")
    aug = nc.dram_tensor("aug", [4 * HPC, S], F32R, kind="ExternalInput")
    tri = nc.dram_tensor("tri", [P, P], F32R, kind="ExternalInput")
    zs = nc.dram_tensor("zs", [62, S], F32R, kind="ExternalInput")
    on1 = nc.dram_tensor("on1", [P, 64], F32R, kind="ExternalInput")
    out = nc.dram_tensor("out", [S, CW], F32, kind="ExternalOutput")

    xT_drams = [qT, kT, vT]
    w_drams = [wqT, wkT, wvT]

    with tile.TileContext(nc) as tc:
        with (
            tc.tile_pool(name="wp", bufs=1) as wp,
            tc.tile_pool(name="xp", bufs=2) as xp,
            tc.tile_pool(name="qk", bufs=1) as qkp,
            tc.tile_pool(name="vv", bufs=1) as vvp,
            tc.tile_pool(name="pt", bufs=3) as ptp,
            tc.tile_pool(name="ot", bufs=2) as otp_sb,
            tc.tile_pool(name="ob", bufs=3) as obp,
            tc.tile_pool(name="rc", bufs=3) as rcp,
            tc.tile_pool(name="cn", bufs=1) as cnp,
        ):
            # ---- constants ----
            tri_t = cnp.tile([P, P], F32R, tag="tri", name="tri_t")
            nc.sync.dma_start(tri_t[:], tri[:])
            ident = cnp.tile([HWID + 1, HWID + 1], F32, tag="ident", name="ident")
            make_identity(nc, ident[:])
            ebias = cnp.tile([P, 1], F32, tag="ebias", name="ebias")
            nc.gpsimd.memset(ebias[:], EXP_BIAS)

            # ---- weights [P, NKC, CW] ----
            w_tiles = []
            for i, wd in enumerate(w_drams):
                wt = wp.tile([P, NKC, CW], F32R, tag=f"w{i}", name=f"w{i}")
                nc.sync.dma_start(wt[:], wd.rearrange("(ko p) c -> p ko c", p=P))
                w_tiles.append(wt)

            # ---- per-head Q~^T / K~^T tiles [128, S]; V~ [128, NJ, HPC, 65] ----
            # even local head: data rows 0:64, aug rows 64:66, matmul slice [0:66]
            # odd  local head: data rows 64:128, aug rows 62:64, slice [62:128]
            q_tiles = [qkp.tile([P, S], F32R, tag=f"qh{h}", name=f"qh{h}") for h in range(HPC)]
            k_tiles = [qkp.tile([P, S], F32R, tag=f"kh{h}", name=f"kh{h}") for h in range(HPC)]
            v_tile = vvp.tile([P, NJ, HPC, HWID + 1], F32R, tag="v", name="v")

            def aug_row(h):
                return HWID if h % 2 == 0 else HWID - 2

            def mm_slice(h):
                return slice(0, 66) if h % 2 == 0 else slice(0, 128)

            for h in range(HPC):
                ra = aug_row(h)
                nc.sync.dma_start(q_tiles[h][ra:ra + 2, :], aug[4 * h:4 * h + 2, :])
                nc.sync.dma_start(k_tiles[h][ra:ra + 2, :], aug[4 * h + 2:4 * h + 4, :])
            for h in range(1, HPC, 2):
                nc.sync.dma_start(q_tiles[h][0:62, :], zs[:])
                nc.sync.dma_start(k_tiles[h][0:62, :], zs[:])
            # ones column of V~
            nc.sync.dma_start(
                v_tile[:, :, :, HWID], on1.rearrange("p (a b) -> p a b", a=NJ)
            )

            # ================= projection phase =================
            with tc.tile_pool(name="pp", bufs=6, space="PSUM") as pp:
                for t in range(NTSB):
                    xt = xp.tile([P, NKC, 3, TSB], F32R, tag="x", name=f"x{t}")
                    for xi, xd in enumerate(xT_drams):
                        nc.sync.dma_start(
                            xt[:, :, xi, :],
                            xd.rearrange("(ko p) t -> p ko t", p=P)[
                                :, :, t * TSB:(t + 1) * TSB],
                        )
                    # Q^T, K^T: [128 (2 heads), TSB] accumulating over d
                    for pi, (wt, dsts) in enumerate(
                        [(w_tiles[0], q_tiles), (w_tiles[1], k_tiles)]
                    ):
                        for cc in range(2):
                            ps = pp.tile([P, TSB], F32, tag="pp", name=f"pp{t}_{pi}_{cc}")
                            for kk in range(NKC):
                                nc.tensor.matmul(
                                    ps[:],
                                    lhsT=wt[:, kk, cc * P:(cc + 1) * P],
                                    rhs=xt[:, kk, pi, :],
                                    start=(kk == 0),
                                    stop=(kk == NKC - 1),
                                )
                            # copyback: rows 0:64 -> head 2cc (rows 0:64),
                            #           rows 64:128 -> head 2cc+1 (rows 64:128)
                            nc.vector.tensor_copy(
                                dsts[2 * cc][0:HWID, t * TSB:(t + 1) * TSB],
                                ps[0:HWID, :],
                            )
                            nc.vector.tensor_copy(
                                dsts[2 * cc + 1][HWID:P, t * TSB:(t + 1) * TSB],
                                ps[HWID:P, :],
                            )
                    # V: [128 t, CW] per 128-t chunk
                    for u in range(TSB // P):
                        tt = t * (TSB // P) + u
                        ps = pp.tile([P, CW], F32, tag="pp", name=f"ppv{t}_{u}")
                        for kk in range(NKC):
                            nc.tensor.matmul(
                                ps[:],
                                lhsT=xt[:, kk, 2, u * P:(u + 1) * P],
                                rhs=w_tiles[2][:, kk, :],
                                start=(kk == 0),
                                stop=(kk == NKC - 1),
                            )
                        nc.vector.tensor_copy(
                            v_tile[:, tt, :, 0:HWID],
                            ps[:].rearrange("p (h w) -> p h w", h=HPC),
                        )

            # ================= attention phase =================
            with (
                tc.tile_pool(name="sc", bufs=2, space="PSUM") as scp,
                tc.tile_pool(name="ov", bufs=3, space="PSUM") as ovp,
                tc.tile_pool(name="tr", bufs=1, space="PSUM") as trp,
            ):
                for h in range(HPC):
                    sl = mm_slice(h)
                    for ip in range(2):  # i-window of 1024 = i-chunks (2ip, 2ip+1)
                        i_base = 1024 * ip
                        jmax = min(8 * ip + 7, NJ - 1)
                        otps = [
                            ovp.tile([HWID + 1, 512], F32, tag="ov", name=f"ov{h}_{ip}_{k2}")
                            for k2 in range(2)
                        ]
                        for J in range(jmax + 1):
                            dp = J - 8 * ip
                            c0 = max(0, 128 * dp)
                            ps = scp.tile([P, 1024], F32, tag="sc", name=f"sc{h}_{ip}_{J}")
                            # scores S^T[j, i] in psum-bank-sized segments
                            seg = c0
                            while seg < 1024:
                                send = min(1024, (seg // 512 + 1) * 512)
                                nc.tensor.matmul(
                                    ps[:, seg:send],
                                    lhsT=k_tiles[h][sl, J * P:(J + 1) * P],
                                    rhs=q_tiles[h][sl, i_base + seg:i_base + send],
                                    start=True,
                                    stop=True,
                                )
                                seg = send
                            pt = ptp.tile([P, 1024], F32R, tag="pt", name=f"pt{h}_{ip}_{J}")
                            nc.scalar.activation(
                                pt[:, c0:1024], ps[:, c0:1024],
                                mybir.ActivationFunctionType.Exp,
                                bias=ebias[:], scale=1.0,
                            )
                            if dp >= 0:
                                # mask the diagonal 128-strip
                                nc.vector.tensor_mul(
                                    pt[:, c0:c0 + P],
                                    pt[:, c0:c0 + P],
                                    tri_t[:],
                                )
                            for ii in range(2):
                                I = 2 * ip + ii
                                i0 = max(512 * I, 128 * J)
                                iend = 512 * I + 512
                                if i0 >= iend:
                                    continue
                                nc.tensor.matmul(
                                    otps[ii][:, i0 - 512 * I:512],
                                    lhsT=v_tile[:, J, h, :],
                                    rhs=pt[:, i0 - i_base:iend - i_base],
                                    start=(J == 0),
                                    stop=(J == min(4 * I + 3, jmax)),
                                )
                        # epilogue: transpose O^T, divide by denominator, store
                        for ii in range(2):
                            I = 2 * ip + ii
                            osb = otp_sb.tile([HWID + 1, 512], F32, tag="ot", name=f"ot{h}_{ip}_{ii}")
                            nc.vector.tensor_copy(osb[:], otps[ii][:])
                            for u in range(4):
                                otr = trp.tile([P, HWID + 1], F32, tag="tr", name=f"tr{h}_{ip}_{ii}_{u}")
                                nc.tensor.transpose(
                                    otr[:], osb[:, u * P:(u + 1) * P], ident[:]
                                )
                                rec = rcp.tile([P, 1], F32, tag="rc", name=f"rc{h}_{ip}_{ii}_{u}")
                                nc.vector.reciprocal(rec[:], otr[:, HWID:HWID + 1])
                                ob = obp.tile([P, HWID], F32, tag="ob", name=f"ob{h}_{ip}_{ii}_{u}")
                                nc.scalar.mul(ob[:], otr[:, 0:HWID], rec[:])
                                nc.sync.dma_start(
                                    out[512 * I + u * P:512 * I + (u + 1) * P,
                                        h * HWID:(h + 1) * HWID],
                                    ob[:],
                                )
    nc.compile()
    return nc


_NC = None


def _get_nc():
    global _NC
    if _NC is None:
        _NC = build_kernel()
    return _NC


def kernel(queries, keys, values, mask, Wq, Wk, Wv):
    B = queries.shape[0]
    asc = np.ascontiguousarray
    scale = 1.0 / np.sqrt(HWID)

    WqTs = asc((Wq.T * scale).astype(np.float32))
    WkT = asc(Wk.T.astype(np.float32))
    WvT = asc(Wv.T.astype(np.float32))
    qTs = [asc(queries[b].T.astype(np.float32)) for b in range(B)]
    kTs = [asc(keys[b].T.astype(np.float32)) for b in range(B)]
    vTs = [asc(values[b].T.astype(np.float32)) for b in range(B)]

    slopes = (2.0 ** (-np.arange(1, H + 1) * (8.0 / H))).astype(np.float32)
    iv = np.arange(S, dtype=np.float32)
    tri_np = np.asarray(
        np.arange(P)[:, None] <= np.arange(P)[None, :], dtype=np.float32
    )  # keep j<=i: rows p (j), cols u (i)

    nc = _get_nc()
    in_maps = []
    for c in range(8):
        b, g = divmod(c, 4)
        a = np.zeros((4 * HPC, S), np.float32)
        for hl in range(HPC):
            h = 4 * g + hl
            a[4 * hl + 0] = slopes[h]
            a[4 * hl + 1] = -slopes[h] * iv
            a[4 * hl + 2] = iv
            a[4 * hl + 3] = 1.0
        in_maps.append({
            "qT": qTs[b], "kT": kTs[b], "vT": vTs[b],
            "wqT": asc(WqTs[:, g * CW:(g + 1) * CW]),
            "wkT": asc(WkT[:, g * CW:(g + 1) * CW]),
            "wvT": asc(WvT[:, g * CW:(g + 1) * CW]),
            "aug": a, "tri": tri_np, "zs": np.zeros((62, S), np.float32), "on1": np.ones((P, 64), np.float32),
        })

    res = run_bass_kernel_spmd(nc, in_maps, core_ids=list(range(8)))
    outp = np.empty((B, S, D), np.float32)
    for c in range(8):
        b, g = divmod(c, 4)
        outp[b, :, g * CW:(g + 1) * CW] = res.results[c]["out"]
    return outp



# revision 2
# speedup vs baseline: 1.4269x; 1.4269x over previous
"""MHSA (dense transformer, ALiBi + causal) TRN2 Bass kernel, 8-core SPMD.

v2 design (vs v1 baseline):
- Sharding: batch (2) x head-quartile -> 8 cores, with heads REBALANCED so
  every core gets one head from each slope quartile: core c (b=c//4, k=c%4)
  computes heads [12+k, 8+k, 4+k, 0+k] (0-indexed) of batch b. This makes the
  per-slot ALiBi band (see below) identical across cores (SPMD-friendly).
- bf16 inputs/weights for the QKV projections (halves input DMA, 1 cyc/row PE).
  Q^T/K^T PSUM results copied to SBUF as f32r (gpsimd/Pool engine), so the
  score matmuls keep exact ALiBi aug rows:
    Q~ = [Q; slope; -slope*i], K~ = [K; j; 1]  (66 contraction rows, f32r)
- Banded causal attention: ALiBi slope*(i-j) > ~40 => weight < e^-40, skipped
  structurally. Per-slot bands in 128-blocks: [16, 16, 6, 3] (slot s covers
  j-blocks J with i_blk - J < B_s).
- Scores S^T[j, i] computed per (slot, J, <=1024-wide i-chunk) into PSUM,
  exp on ACT engine -> P^T in bf16, diagonal 128-strip masked by a 0/1
  lower-triangular multiply (DVE).
- AV matmul REORIENTED: out[i(128), 65] = P^T-chunkT @ V~ (lhsT = P^T bf16,
  rhs = V~ bf16 with a ones column), accumulated over J in PSUM. Column 64 is
  the softmax denominator. No PE transposes needed; epilogue = batched
  reciprocal + per-block tensor_scalar multiply (DVE) into an SBUF output
  staging tile, DMA'd out once per slot.
- Software pipelining: scores/exp of strip n are emitted before the AV of
  strip n-1 so PE never stalls on ACT.
- qs tiles padded to 2176 cols so f32r score segments are always >=256 wide
  (1 cyc/row).
"""

import numpy as np
import ml_dtypes

import concourse.bass as bass
import concourse.mybir as mybir
import concourse.tile as tile
from concourse import bacc
from concourse.bass_utils import run_bass_kernel_spmd

P = 128
S = 2048
SPAD = S + 128    # padded i extent of qs tiles
D = 1024
H = 16
HWID = 64
HPC = 4           # head slots per core
CW = HPC * HWID   # 256
NKC = D // P      # 8 contraction chunks
NTSB = 4          # t super-blocks for projection streaming
TSB = S // NTSB   # 512
NJ = S // P       # 16
AUG = 2
QROWS = HWID + AUG  # 66
BANDS = [16, 16, 6, 3]      # causal band per slot, in 128-blocks
SLOT_BASE = [12, 8, 4, 0]   # head (0-indexed) = SLOT_BASE[s] + (core % 4)
ACC_GRP = 6                 # i-blocks per PSUM accumulator tile (65 cols + pad 80)

F32 = mybir.dt.float32
F32R = mybir.dt.float32r
BF16 = mybir.dt.bfloat16

EXP_BIAS = -12.0
BF16NP = ml_dtypes.bfloat16


def build_kernel():
    nc = bacc.Bacc("TRN2")

    xq = nc.dram_tensor("xq", [D, S], BF16, kind="ExternalInput")
    xk = nc.dram_tensor("xk", [D, S], BF16, kind="ExternalInput")
    xv = nc.dram_tensor("xv", [D, S], BF16, kind="ExternalInput")
    wq = nc.dram_tensor("wq", [D, CW], BF16, kind="ExternalInput")
    wk = nc.dram_tensor("wk", [D, CW], BF16, kind="ExternalInput")
    wv = nc.dram_tensor("wv", [D, CW], BF16, kind="ExternalInput")
    augq = nc.dram_tensor("augq", [HPC, AUG, SPAD], F32R, kind="ExternalInput")
    augk = nc.dram_tensor("augk", [AUG, S], F32R, kind="ExternalInput")
    tri = nc.dram_tensor("tri", [P, P], BF16, kind="ExternalInput")
    on1 = nc.dram_tensor("on1", [P, NJ * HPC], BF16, kind="ExternalInput")
    out = nc.dram_tensor("out", [S, CW], F32, kind="ExternalOutput")

    x_drams = [xq, xk, xv]
    w_drams = [wq, wk, wv]

    with tile.TileContext(nc) as tc:
        with (
            tc.tile_pool(name="cn", bufs=1) as cnp,
            tc.tile_pool(name="wp", bufs=1) as wp,
            tc.tile_pool(name="qk", bufs=1) as qkp,
            tc.tile_pool(name="vv", bufs=1) as vvp,
            tc.tile_pool(name="xp", bufs=2) as xp,
            tc.tile_pool(name="pt", bufs=3) as ptp,
            tc.tile_pool(name="rc", bufs=3) as rcp,
            tc.tile_pool(name="ob", bufs=1) as obp,
        ):
            # ---- constants ----
            tri_t = cnp.tile([P, P], BF16, tag="tri", name="tri_t")
            nc.sync.dma_start(tri_t[:], tri[:])
            ebias = cnp.tile([P, 1], F32, tag="ebias", name="ebias")
            nc.gpsimd.memset(ebias[:], EXP_BIAS)

            # ---- weights [P, NKC, CW] bf16 ----
            w_tiles = []
            for i, wd in enumerate(w_drams):
                wt = wp.tile([P, NKC, CW], BF16, tag=f"w{i}", name=f"w{i}")
                nc.sync.dma_start(wt[:], wd.rearrange("(ko p) c -> p ko c", p=P))
                w_tiles.append(wt)

            # ---- per-slot Q~^T [66, SPAD] / K~^T [66, S] f32r; V~ bf16 ----
            qs = [qkp.tile([QROWS, SPAD], F32R, tag=f"q{s}", name=f"q{s}")
                  for s in range(HPC)]
            ks = [qkp.tile([QROWS, S], F32R, tag=f"k{s}", name=f"k{s}")
                  for s in range(HPC)]
            v_t = vvp.tile([P, NJ, HPC, HWID + 1], BF16, tag="v", name="v_t")

            for s in range(HPC):
                nc.sync.dma_start(qs[s][HWID:QROWS, :], augq[s])
                nc.sync.dma_start(ks[s][HWID:QROWS, :], augk[:])
                # zero the padded i columns of the data rows
                nc.gpsimd.memset(qs[s][0:HWID, S:SPAD], 0.0)
            nc.sync.dma_start(
                v_t[:, :, :, HWID], on1.rearrange("p (a b) -> p a b", a=NJ)
            )

            # ---- output staging [P, NJ, CW] f32 ----
            out_sb = obp.tile([P, NJ, CW], F32, tag="ob", name="out_sb")

            # ================= projection phase =================
            with (
                tc.tile_pool(name="pq", bufs=4, space="PSUM") as pqp,
                tc.tile_pool(name="pv", bufs=2, space="PSUM") as pvp,
            ):
                for t in range(NTSB):
                    xt = xp.tile([P, NKC, 3, TSB], BF16, tag="x", name=f"x{t}")
                    for xi, xd in enumerate(x_drams):
                        nc.sync.dma_start(
                            xt[:, :, xi, :],
                            xd.rearrange("(ko p) t -> p ko t", p=P)[
                                :, :, t * TSB:(t + 1) * TSB],
                        )
                    # Q^T / K^T: psum [128 (2 slots), TSB], accumulate over d
                    for pi in range(2):
                        dsts = qs if pi == 0 else ks
                        for cc in range(2):
                            ps = pqp.tile([P, TSB], F32, tag="pq",
                                          name=f"pq{t}_{pi}_{cc}")
                            for kk in range(NKC):
                                nc.tensor.matmul(
                                    ps[:],
                                    lhsT=w_tiles[pi][:, kk, cc * P:(cc + 1) * P],
                                    rhs=xt[:, kk, pi, :],
                                    start=(kk == 0),
                                    stop=(kk == NKC - 1),
                                )
                            nc.gpsimd.tensor_copy(
                                dsts[2 * cc][0:HWID, t * TSB:(t + 1) * TSB],
                                ps[0:HWID, :],
                            )
                            nc.gpsimd.tensor_copy(
                                dsts[2 * cc + 1][0:HWID, t * TSB:(t + 1) * TSB],
                                ps[HWID:P, :],
                            )
                    # V: psum [128 t, CW] per 128-t chunk -> bf16 v_t
                    for u in range(TSB // P):
                        tt = t * (TSB // P) + u
                        ps = pvp.tile([P, CW], F32, tag="pv", name=f"pv{t}_{u}")
                        for kk in range(NKC):
                            nc.tensor.matmul(
                                ps[:],
                                lhsT=xt[:, kk, 2, u * P:(u + 1) * P],
                                rhs=w_tiles[2][:, kk, :],
                                start=(kk == 0),
                                stop=(kk == NKC - 1),
                            )
                        nc.vector.tensor_copy(
                            v_t[:, tt, :, 0:HWID],
                            ps[:].rearrange("p (h w) -> p h w", h=HPC),
                        )

            # ================= attention phase =================
            # strip list: (slot, J, i0, i1); i-chunks of <=1024
            strips = []
            for s in range(HPC):
                B = BANDS[s]
                for J in range(NJ):
                    lo = P * J
                    hi = min(S, P * (J + B))
                    c = lo
                    while c < hi:
                        ce = min(hi, c + 1024)
                        strips.append((s, J, c, ce))
                        c = ce

            with (
                tc.tile_pool(name="sc", bufs=2, space="PSUM") as scp,
                tc.tile_pool(name="av", bufs=1, space="PSUM") as avp,
            ):
                acc = [avp.tile([P, ACC_GRP, 80], F32, tag=f"ac{q}",
                                name=f"ac{q}") for q in range(3)]
                pts = {}

                def emit_scores_exp(n):
                    s, J, i0, i1 = strips[n]
                    W = i1 - i0
                    sc = scp.tile([P, 1024], F32, tag="sc", name=f"sc{n}")
                    seg = 0
                    while seg < W:
                        send = min(seg + 512, W)
                        if send - seg < 256:
                            send = seg + 256  # pad into qs junk cols (zeroed)
                        nc.tensor.matmul(
                            sc[:, seg:send],
                            lhsT=ks[s][0:QROWS, P * J:P * (J + 1)],
                            rhs=qs[s][0:QROWS, i0 + seg:i0 + send],
                            start=True,
                            stop=True,
                        )
                        seg = send
                    pt = ptp.tile([P, 1024], BF16, tag="pt", name=f"pt{n}")
                    nc.scalar.activation(
                        pt[:, 0:W], sc[:, 0:W],
                        mybir.ActivationFunctionType.Exp,
                        bias=ebias[:], scale=1.0,
                    )
                    if i0 == P * J:
                        # mask the diagonal 128-strip (keep j <= i)
                        nc.vector.tensor_mul(pt[:, 0:P], pt[:, 0:P], tri_t[:])
                    pts[n] = pt

                def emit_av(n):
                    s, J, i0, i1 = strips[n]
                    B = BANDS[s]
                    pt = pts.pop(n)
                    for i_blk in range(i0 // P, i1 // P):
                        col = i_blk * P - i0
                        q, r = divmod(i_blk, ACC_GRP)
                        nc.tensor.matmul(
                            acc[q][:, r, 0:HWID + 1],
                            lhsT=pt[:, col:col + P],
                            rhs=v_t[:, J, s, :],
                            start=(J == max(0, i_blk - B + 1)),
                            stop=(J == i_blk),
                        )
                        if J == i_blk and (r == ACC_GRP - 1 or i_blk == NJ - 1):
                            # acc tile q complete: normalize its i-blocks
                            nr = r + 1
                            rec = rcp.tile([P, ACC_GRP, 1], F32, tag="rc",
                                           name=f"rc{s}_{q}")
                            nc.vector.reciprocal(
                                rec[:, 0:nr, :],
                                acc[q][:, 0:nr, HWID:HWID + 1],
                            )
                            for rr in range(nr):
                                ib = q * ACC_GRP + rr
                                nc.vector.tensor_scalar_mul(
                                    out_sb[:, ib, s * HWID:(s + 1) * HWID],
                                    acc[q][:, rr, 0:HWID],
                                    rec[:, rr, :],
                                )
                            if i_blk == NJ - 1:
                                # slot finished: stream its output columns out
                                nc.sync.dma_start(
                                    out.rearrange("(a p) c -> p a c", p=P)[
                                        :, :, s * HWID:(s + 1) * HWID],
                                    out_sb[:, :, s * HWID:(s + 1) * HWID],
                                )

                for n in range(len(strips)):
                    emit_scores_exp(n)
                    if n > 0:
                        emit_av(n - 1)
                emit_av(len(strips) - 1)

    nc.compile()
    return nc


_NC = None


def _get_nc():
    global _NC
    if _NC is None:
        _NC = build_kernel()
    return _NC


def kernel(queries, keys, values, mask, Wq, Wk, Wv):
    B = queries.shape[0]
    asc = np.ascontiguousarray
    scale = 1.0 / np.sqrt(HWID)

    WqT = asc((np.asarray(Wq).T * scale).astype(np.float32)).astype(BF16NP)
    WkT = asc(np.asarray(Wk).T.astype(np.float32)).astype(BF16NP)
    WvT = asc(np.asarray(Wv).T.astype(np.float32)).astype(BF16NP)
    qTs = [asc(np.asarray(queries[b]).T.astype(np.float32)).astype(BF16NP)
           for b in range(B)]
    kTs = [asc(np.asarray(keys[b]).T.astype(np.float32)).astype(BF16NP)
           for b in range(B)]
    vTs = [asc(np.asarray(values[b]).T.astype(np.float32)).astype(BF16NP)
           for b in range(B)]

    slopes = (2.0 ** (-np.arange(1, H + 1) * (8.0 / H))).astype(np.float32)
    ipad = np.arange(SPAD, dtype=np.float32)
    augk_np = np.stack([np.arange(S, dtype=np.float32),
                        np.ones(S, np.float32)])
    tri_np = np.asarray(
        np.arange(P)[:, None] <= np.arange(P)[None, :], dtype=np.float32
    ).astype(BF16NP)  # keep j<=i: rows p (j), cols u (i)
    on1_np = np.ones((P, NJ * HPC), BF16NP)

    nc = _get_nc()
    in_maps = []
    for c in range(8):
        b, k = divmod(c, 4)
        heads = [SLOT_BASE[s] + k for s in range(HPC)]
        aq = np.zeros((HPC, AUG, SPAD), np.float32)
        for s, h in enumerate(heads):
            aq[s, 0, :] = slopes[h]
            aq[s, 1, :] = -slopes[h] * ipad
        cols = np.concatenate(
            [np.arange(h * HWID, (h + 1) * HWID) for h in heads])
        in_maps.append({
            "xq": qTs[b], "xk": kTs[b], "xv": vTs[b],
            "wq": asc(WqT[:, cols]), "wk": asc(WkT[:, cols]),
            "wv": asc(WvT[:, cols]),
            "augq": aq, "augk": augk_np, "tri": tri_np, "on1": on1_np,
        })

    res = run_bass_kernel_spmd(nc, in_maps, core_ids=list(range(8)))
    outp = np.empty((B, S, D), np.float32)
    for c in range(8):
        b, k = divmod(c, 4)
        for s in range(HPC):
            h = SLOT_BASE[s] + k
            outp[b, :, h * HWID:(h + 1) * HWID] = \
                res.results[c]["out"][:, s * HWID:(s + 1) * HWID]
    return outp


# revision 4
# speedup vs baseline: 1.7636x; 1.2359x over previous
"""MHSA (dense transformer, ALiBi + causal) TRN2 Bass kernel, 8-core SPMD.

v4 design:
- Sharding: batch (2) x head-quartile -> 8 cores, heads REBALANCED so every
  core gets one head from each ALiBi-slope quartile: core c (b=c//4, k=c%4)
  computes heads [12+k, 8+k, 4+k, 0+k] (0-indexed) of batch b. Slot s on all
  cores then shares one causal band -> SPMD-friendly block skipping.
- Banded causal attention: ALiBi slope*(i-j) >~ 32 => weight < e^-32,
  skipped structurally. Bands (in 128-blocks) per slot: [16, 16, 5, 2].
- All matmuls bf16 (1 cyc/row on PE at any width). ALiBi folded into 3
  bf16-exact aug contraction rows:
    Q~ = [Q; slope; slope; -slope*i],  K~ = [K; j_hi; j_lo; 1]
  with j_hi multiple of 256 and j_lo in [0,256) - both exact in bf16; the
  -slope*i row is a per-row shift that cancels in softmax.
- Projection phase (x/w bf16, PSUM f32): Q^T/K^T copied to bf16 SBUF slot
  tiles by DVE, V by DVE into a [j, slot, 65] bf16 tile with a ones column
  (column 64 of the AV output becomes the softmax denominator).
- Attention interleaved with projections by i-window: window t = i in
  [512t, 512t+512), its strip groups round-robined with the projection
  chains of superblock t+1 so ACT exp overlaps PE projection matmuls.
- Scores S^T[j,i] per (slot, J): strips grouped into <=1024-col PSUM tiles,
  ONE exp (ACT) per group -> bf16 P^T; diagonal 128-strips masked on Pool
  (gpsimd) with a 0/1 triangle.
- AV: out[i(128), 65] accumulated over J into a [128, 4, 80] PSUM tile.
  IMPORTANT: PSUM accumulation groups are tracked per 2KB bank - only ONE
  chain may be open per bank at a time (interleaved starts lazily re-zero
  the bank). So all AV chains of a window are emitted at window END, one
  i-block at a time, each chain fully closed before the next starts.
- Epilogue = batched reciprocal + tensor_scalar multiplies (DVE) into an
  SBUF staging tile, DMA'd out per slot-pair (512B elements).
"""

import numpy as np
import ml_dtypes

import concourse.bass as bass
import concourse.mybir as mybir
import concourse.tile as tile
from concourse import bacc
from concourse.bass_utils import run_bass_kernel_spmd

P = 128
S = 2048
D = 1024
H = 16
HWID = 64
HPC = 4           # head slots per core
CW = HPC * HWID   # 256
NKC = D // P      # 8 contraction chunks
NTSB = 4          # t super-blocks (projection + attention i-windows)
TSB = S // NTSB   # 512
NJ = S // P       # 16
AUG = 3
QROWS = HWID + AUG  # 67
BANDS = [16, 16, 5, 2]      # causal band per slot, in 128-blocks
SLOT_BASE = [12, 8, 4, 0]   # head (0-indexed) = SLOT_BASE[s] + (core % 4)

F32 = mybir.dt.float32
BF16 = mybir.dt.bfloat16

EXP_BIAS = -12.0
BF16NP = ml_dtypes.bfloat16


def window_strips(s, t):
    """Strips (J, i0, i1) of window t for slot s (banded causal)."""
    B = BANDS[s]
    res = []
    for J in range(max(0, 4 * t - B + 1), 4 * t + 4):
        i0 = max(TSB * t, P * J)
        i1 = min(TSB * t + TSB, P * (J + B), S)
        if i1 > i0:
            res.append((J, i0, i1))
    return res


def group_strips(strips_w, cap=1024):
    groups, cur, w = [], [], 0
    for (J, i0, i1) in strips_w:
        if w + (i1 - i0) > cap and cur:
            groups.append(cur)
            cur, w = [], 0
        cur.append((J, i0, i1))
        w += i1 - i0
    if cur:
        groups.append(cur)
    return groups


def build_kernel():
    nc = bacc.Bacc("TRN2")

    xq = nc.dram_tensor("xq", [D, S], BF16, kind="ExternalInput")
    xk = nc.dram_tensor("xk", [D, S], BF16, kind="ExternalInput")
    xv = nc.dram_tensor("xv", [D, S], BF16, kind="ExternalInput")
    wq = nc.dram_tensor("wq", [D, CW], BF16, kind="ExternalInput")
    wk = nc.dram_tensor("wk", [D, CW], BF16, kind="ExternalInput")
    wv = nc.dram_tensor("wv", [D, CW], BF16, kind="ExternalInput")
    augq = nc.dram_tensor("augq", [HPC, AUG, S], BF16, kind="ExternalInput")
    augk = nc.dram_tensor("augk", [AUG, S], BF16, kind="ExternalInput")
    tri = nc.dram_tensor("tri", [P, P], BF16, kind="ExternalInput")
    on1 = nc.dram_tensor("on1", [P, NJ * HPC], BF16, kind="ExternalInput")
    out = nc.dram_tensor("out", [S, CW], F32, kind="ExternalOutput")

    x_drams = [xq, xk, xv]
    w_drams = [wq, wk, wv]

    with tile.TileContext(nc) as tc:
        with (
            tc.tile_pool(name="cn", bufs=1) as cnp,
            tc.tile_pool(name="wp", bufs=1) as wp,
            tc.tile_pool(name="qk", bufs=1) as qkp,
            tc.tile_pool(name="vv", bufs=1) as vvp,
            tc.tile_pool(name="xp", bufs=2) as xp,
            tc.tile_pool(name="pt", bufs=12) as ptp,
            tc.tile_pool(name="rc", bufs=3) as rcp,
            tc.tile_pool(name="ob", bufs=1) as obp,
            tc.tile_pool(name="pq", bufs=2, space="PSUM") as pqp,
            tc.tile_pool(name="sc", bufs=2, space="PSUM") as scp,
            tc.tile_pool(name="av", bufs=2, space="PSUM") as avp,
        ):
            # ---- weights first (critical path), then x(0), then consts ----
            w_tiles = []
            for i, wd in enumerate(w_drams):
                wt = wp.tile([P, NKC, CW], BF16, tag=f"w{i}", name=f"w{i}")
                nc.sync.dma_start(wt[:], wd.rearrange("(ko p) c -> p ko c", p=P))
                w_tiles.append(wt)

            def emit_xdma(t):
                xt = xp.tile([P, NKC, 3, TSB], BF16, tag="x", name=f"x{t}")
                for xi, xd in enumerate(x_drams):
                    nc.sync.dma_start(
                        xt[:, :, xi, :],
                        xd.rearrange("(ko p) t -> p ko t", p=P)[
                            :, :, t * TSB:(t + 1) * TSB],
                    )
                return xt

            xt0 = emit_xdma(0)

            # ---- constants ----
            tri_t = cnp.tile([P, P], BF16, tag="tri", name="tri_t")
            nc.sync.dma_start(tri_t[:], tri[:])
            ebias = cnp.tile([P, 1], F32, tag="ebias", name="ebias")
            nc.gpsimd.memset(ebias[:], EXP_BIAS)

            # ---- per-slot Q~^T / K~^T [67, S] bf16; V~ [128, NJ, HPC, 65] ----
            qs = [qkp.tile([QROWS, S], BF16, tag=f"q{s}", name=f"q{s}")
                  for s in range(HPC)]
            ks = [qkp.tile([QROWS, S], BF16, tag=f"k{s}", name=f"k{s}")
                  for s in range(HPC)]
            v_t = vvp.tile([P, NJ, HPC, HWID + 1], BF16, tag="v", name="v_t")

            for s in range(HPC):
                nc.sync.dma_start(qs[s][HWID:QROWS, :], augq[s])
                nc.sync.dma_start(ks[s][HWID:QROWS, :], augk[:])
            nc.sync.dma_start(
                v_t[:, :, :, HWID], on1.rearrange("p (a b) -> p a b", a=NJ)
            )

            # ---- output staging [P, NJ, CW] f32 ----
            out_sb = obp.tile([P, NJ, CW], F32, tag="ob", name="out_sb")

            # ---------- emission helpers ----------
            def proj_chain_units(t, xt):
                units = []
                for pi in range(2):
                    for cc in range(2):
                        units.append(("qk", t, xt, pi, cc))
                for u in range(TSB // P):
                    units.append(("v", t, xt, u))
                return units

            def emit_chain(unit):
                kind = unit[0]
                if kind == "qk":
                    _, t, xt, pi, cc = unit
                    dsts = qs if pi == 0 else ks
                    ps = pqp.tile([P, TSB], F32, tag="pq",
                                  name=f"pq{t}_{pi}_{cc}")
                    for kk in range(NKC):
                        nc.tensor.matmul(
                            ps[:],
                            lhsT=w_tiles[pi][:, kk, cc * P:(cc + 1) * P],
                            rhs=xt[:, kk, pi, :],
                            start=(kk == 0),
                            stop=(kk == NKC - 1),
                        )
                    nc.vector.tensor_copy(
                        dsts[2 * cc][0:HWID, t * TSB:(t + 1) * TSB],
                        ps[0:HWID, :],
                    )
                    nc.vector.tensor_copy(
                        dsts[2 * cc + 1][0:HWID, t * TSB:(t + 1) * TSB],
                        ps[HWID:P, :],
                    )
                else:
                    _, t, xt, u = unit
                    tt = t * (TSB // P) + u
                    ps = pqp.tile([P, TSB], F32, tag="pq", name=f"pv{t}_{u}")
                    for kk in range(NKC):
                        nc.tensor.matmul(
                            ps[:, 0:CW],
                            lhsT=xt[:, kk, 2, u * P:(u + 1) * P],
                            rhs=w_tiles[2][:, kk, :],
                            start=(kk == 0),
                            stop=(kk == NKC - 1),
                        )
                    nc.vector.tensor_copy(
                        v_t[:, tt, :, 0:HWID],
                        ps[:, 0:CW].rearrange("p (h w) -> p h w", h=HPC),
                    )

            def emit_scores_exp(s, t, g, gi):
                width = sum(i1 - i0 for (_, i0, i1) in g)
                sc = scp.tile([P, 1024], F32, tag="sc", name=f"sc{s}_{t}_{gi}")
                o = 0
                offs = []
                for (J, i0, i1) in g:
                    W = i1 - i0
                    a = 0
                    while a < W:  # split at psum bank boundaries (512 cols)
                        b = min(W, a + 512 - (o + a) % 512)
                        nc.tensor.matmul(
                            sc[:, o + a:o + b],
                            lhsT=ks[s][0:QROWS, P * J:P * (J + 1)],
                            rhs=qs[s][0:QROWS, i0 + a:i0 + b],
                            start=True,
                            stop=True,
                        )
                        a = b
                    offs.append((J, i0, i1, o))
                    o += W
                pt = ptp.tile([P, 1024], BF16, tag="pt", name=f"pt{s}_{t}_{gi}")
                nc.scalar.activation(
                    pt[:, 0:width], sc[:, 0:width],
                    mybir.ActivationFunctionType.Exp,
                    bias=ebias[:], scale=1.0,
                )
                for (J, i0, i1, off) in offs:
                    if i0 == P * J:
                        # mask the diagonal 128-strip (keep j <= i), on Pool
                        nc.gpsimd.tensor_mul(
                            pt[:, off:off + P], pt[:, off:off + P], tri_t[:]
                        )
                return pt, offs

            def emit_epilogue(s, t, acc):
                rec = rcp.tile([P, 4, 1], F32, tag="rc", name=f"rc{s}_{t}")
                nc.vector.reciprocal(rec[:], acc[:, :, HWID:HWID + 1])
                for r in range(4):
                    nc.vector.tensor_scalar_mul(
                        out_sb[:, 4 * t + r, s * HWID:(s + 1) * HWID],
                        acc[:, r, 0:HWID],
                        rec[:, r, :],
                    )
                if t == NTSB - 1 and s % 2 == 1:
                    # slot pair done: 128 contiguous f32 columns -> 512B elems
                    nc.sync.dma_start(
                        out.rearrange("(a p) c -> p a c", p=P)[
                            :, :, (s - 1) * HWID:(s + 1) * HWID],
                        out_sb[:, :, (s - 1) * HWID:(s + 1) * HWID],
                    )

            # pending completed window: (s, t, acc, [(pt, offs), ...])
            pend_w = None

            def flush_window():
                nonlocal pend_w
                if pend_w is None:
                    return
                s, t, acc, recs = pend_w
                B = BANDS[s]
                jmap = {}
                for pt, offs in recs:
                    for (J, i0, i1, off) in offs:
                        jmap[J] = (pt, off, i0)
                # one fully-closed accumulation chain per i-block (PSUM bank
                # allows only one open chain at a time)
                for r in range(4):
                    i_blk = 4 * t + r
                    jst = max(0, i_blk - B + 1)
                    for J in range(jst, i_blk + 1):
                        pt, off, i0 = jmap[J]
                        col = off + i_blk * P - i0
                        nc.tensor.matmul(
                            acc[:, r, 0:HWID + 1],
                            lhsT=pt[:, col:col + P],
                            rhs=v_t[:, J, s, :],
                            start=(J == jst),
                            stop=(J == i_blk),
                        )
                emit_epilogue(s, t, acc)
                pend_w = None

            # ---------- main schedule ----------
            for unit in proj_chain_units(0, xt0):
                emit_chain(unit)

            for t in range(NTSB):
                if t + 1 < NTSB:
                    xt_next = emit_xdma(t + 1)
                    next_chains = proj_chain_units(t + 1, xt_next)
                else:
                    next_chains = []
                gitems = []
                for s in range(HPC):
                    gitems.append((s, group_strips(window_strips(s, t))))
                total_groups = sum(len(g) for _, g in gitems)
                ci = 0
                gcount = 0
                for s, groups in gitems:
                    acc = avp.tile([P, 4, 80], F32, tag="acc",
                                   name=f"acc{s}_{t}")
                    recs = []
                    for gi, g in enumerate(groups):
                        pt, offs = emit_scores_exp(s, t, g, gi)
                        recs.append((pt, offs))
                        flush_window()
                        gcount += 1
                        while (ci < len(next_chains)
                               and ci < (gcount * len(next_chains))
                               // total_groups):
                            emit_chain(next_chains[ci])
                            ci += 1
                    pend_w = (s, t, acc, recs)
                while ci < len(next_chains):
                    emit_chain(next_chains[ci])
                    ci += 1
            flush_window()

    nc.compile()
    return nc


_NC = None


def _get_nc():
    global _NC
    if _NC is None:
        _NC = build_kernel()
    return _NC


def kernel(queries, keys, values, mask, Wq, Wk, Wv):
    B = queries.shape[0]
    asc = np.ascontiguousarray
    scale = 1.0 / np.sqrt(HWID)

    WqT = asc((np.asarray(Wq).T * scale).astype(np.float32)).astype(BF16NP)
    WkT = asc(np.asarray(Wk).T.astype(np.float32)).astype(BF16NP)
    WvT = asc(np.asarray(Wv).T.astype(np.float32)).astype(BF16NP)
    qTs = [asc(np.asarray(queries[b]).T.astype(np.float32)).astype(BF16NP)
           for b in range(B)]
    kTs = [asc(np.asarray(keys[b]).T.astype(np.float32)).astype(BF16NP)
           for b in range(B)]
    vTs = [asc(np.asarray(values[b]).T.astype(np.float32)).astype(BF16NP)
           for b in range(B)]

    slopes = (2.0 ** (-np.arange(1, H + 1) * (8.0 / H))).astype(np.float32)
    slopes_bf = slopes.astype(BF16NP).astype(np.float32)
    iv = np.arange(S, dtype=np.float32)
    j_hi = (np.arange(S) // 256 * 256).astype(np.float32)
    j_lo = (np.arange(S) % 256).astype(np.float32)
    augk_np = np.stack([j_hi, j_lo, np.ones(S, np.float32)]).astype(BF16NP)
    tri_np = np.asarray(
        np.arange(P)[:, None] <= np.arange(P)[None, :], dtype=np.float32
    ).astype(BF16NP)  # keep j<=i: rows p (j), cols u (i)
    on1_np = np.ones((P, NJ * HPC), BF16NP)

    nc = _get_nc()
    in_maps = []
    for c in range(8):
        b, k = divmod(c, 4)
        heads = [SLOT_BASE[s] + k for s in range(HPC)]
        aq = np.zeros((HPC, AUG, S), np.float32)
        for s, h in enumerate(heads):
            aq[s, 0, :] = slopes_bf[h]
            aq[s, 1, :] = slopes_bf[h]
            aq[s, 2, :] = -slopes_bf[h] * iv
        cols = np.concatenate(
            [np.arange(h * HWID, (h + 1) * HWID) for h in heads])
        in_maps.append({
            "xq": qTs[b], "xk": kTs[b], "xv": vTs[b],
            "wq": asc(WqT[:, cols]), "wk": asc(WkT[:, cols]),
            "wv": asc(WvT[:, cols]),
            "augq": aq.astype(BF16NP), "augk": augk_np,
            "tri": tri_np, "on1": on1_np,
        })

    res = run_bass_kernel_spmd(nc, in_maps, core_ids=list(range(8)))
    outp = np.empty((B, S, D), np.float32)
    for c in range(8):
        b, k = divmod(c, 4)
        for s in range(HPC):
            h = SLOT_BASE[s] + k
            outp[b, :, h * HWID:(h + 1) * HWID] = \
                res.results[c]["out"][:, s * HWID:(s + 1) * HWID]
    return outp
